# revision 2
# baseline (speedup 1.0000x reference)
"""Trainium2 Bass kernel for batched single-head attention + output projection + layernorm.

Reference computation (per batch element b):
    q = Q@Wq + bq ; k = K@Wk + bk ; v = V@Wv + bv
    S = q k^T / sqrt(DV) ; S[pad_mask==0] = -1e9 ; P = softmax(S)
    out = LN(P v @ Wo + bo; g0, beta0)

Sharding: data-parallel over batch B=8 across the 8 NeuronCores (one batch
element per core, no collectives).

Math folds (exact):
  - q/k only feed the scores: S = (QWq+bq)(KWk+bk)^T. The bk terms are
    constant per query row -> softmax invariant -> drop. Fold
    Wqk = Wq @ Wk^T (host, param-only) so S = Q Wqk K^T + (K @ (Wk bq))^T.
    The bq term is a per-key bias folded into the exp bias below. This
    removes the K projection matmul entirely.
  - softmax rows sum to 1, so out = P (V Wv + bv) Wo + bo
    = P V (Wv Wo) + (bv Wo + bo). Fold Wvo = Wv @ Wo and
    boe = bv @ Wo + bo (host, param-only). Removes the V projection.
  - pad_mask zeroes keys *exactly*: exp(-1e5 + s) underflows to 0 in f32,
    matching the reference's softmax(where(mask==0, -1e9)) which also
    yields exactly-0 weights. So masked keys are gathered away on the
    host: K/V are repacked to only the active keys (padded with zero
    rows + -1e5 bias to a multiple of 128 shared by all cores). With a
    ~50% mask this halves the two NK-sized matmuls.
  - softmax normalization is deferred: O_unnorm = E@V with E = exp(S'),
    normalized by the row-sum computed with a ones-matmul (partition-
    replicated), applied when copying O out of PSUM.
  - scale 1/sqrt(DV) and the per-key bias fuse into the Exp activation:
    E^T = exp(S^T * scale + mbias[j]).

Layout strategy: host passes Q^T/K^T (bf16, key-gathered) and V natural
(key-gathered) so every matmul has its contraction dim on SBUF
partitions; attention runs in transposed score layout (S^T[j,i])
end-to-end, which makes the per-key bias a per-partition bias and
produces the final output in natural [token, feature] layout with zero
on-device transposes.

NOTE: if some batch row has *zero* active keys the reference degenerates
to a uniform softmax over all 2048 keys; this kernel does not reproduce
that (probability 2^-2048 under the spec's random mask).
"""

import numpy as np
import ml_dtypes

import concourse.bass as bass
import concourse.bacc as bacc
import concourse.tile as tile
from concourse import mybir
from concourse.bass_utils import run_bass_kernel_spmd

BF16 = mybir.dt.bfloat16
F32 = mybir.dt.float32
AF = mybir.ActivationFunctionType
P = 128
N_CORES = 8
EPS = 1e-5

# Full-problem shapes (hardcoded; the grading harness runs kernel() standalone).
B, NQ, NK, DQ, DV = 8, 2048, 2048, 1024, 1024


def attention_body(tc, outs, ins, nkp, blk=512):
    nc = tc.nc
    qt, kt, v = ins["qt"], ins["kt"], ins["v"]
    mb = ins["mb"]
    wqk, wvo = ins["wqk"], ins["wvo"]
    boe, g0, b0 = ins["boe"], ins["g0"], ins["b0"]
    out = outs["out"]

    DQ_, NQ_ = qt.shape
    DV_ = wvo.shape[1]
    C = DQ_ // P          # input-feature 128-chunks (contraction of q proj)
    D = DV_ // P          # projected-feature 128-chunks
    JS = nkp // P         # active-key 128-chunks
    IW = min(blk, NQ_)    # query block width (psum free dim)
    EW = min(blk, DV_)    # feature block width
    BW = min(512, DV_)    # bn_stats chunk width
    NB = DV_ // BW        # bn_stats chunks
    PSB = 8               # psum slots (8 banks total)
    NI = NQ_ // IW        # query blocks
    NE = DV_ // EW        # feature blocks
    IS = IW // P          # query 128-chunks per query block
    scale = float(DV_) ** -0.5

    with tc.tile_pool(name="sb", bufs=1) as sb, \
         tc.tile_pool(name="psp", bufs=1, space="PSUM") as psp:

        # ---------------- constants + resident tensors ----------------
        ones = sb.tile([P, P], BF16, tag="ones", bufs=1, name="ones")
        nc.vector.memset(ones, 1.0)
        eps_sb = sb.tile([P, 1], F32, tag="eps", bufs=1, name="eps_sb")
        nc.vector.memset(eps_sb, EPS)
        mb_sb = sb.tile([P, JS], F32, tag="mb", bufs=1, name="mb_sb")
        nc.gpsimd.dma_start(out=mb_sb, in_=mb.rearrange("(j p) -> p j", p=P))

        def bcast(ap, nm):
            t = sb.tile([P, DV_], F32, tag=nm, bufs=1, name=nm)
            nc.gpsimd.dma_start(
                out=t,
                in_=bass.AP(tensor=ap.tensor, offset=ap.offset,
                            ap=[[0, P]] + [list(a) for a in ap.ap]),
            )
            return t

        boe_b = bcast(boe, "boe_b")
        g0_b = bcast(g0, "g0_b")
        b0_b = bcast(b0, "b0_b")

        wqk_sb = sb.tile([P, C, DV_], BF16, tag="wqk", bufs=1, name="wqk_sb")
        for c in range(C):
            nc.sync.dma_start(out=wqk_sb[:, c, :], in_=wqk[c * P:(c + 1) * P, :])
        wvo_sb = sb.tile([P, D, DV_], BF16, tag="wvo", bufs=1, name="wvo_sb")
        for d in range(D):
            nc.sync.dma_start(out=wvo_sb[:, d, :], in_=wvo[d * P:(d + 1) * P, :])

        # K^T resident: kt_sb[d] is [128(feat), nkp] bf16 (raw, no projection)
        kt_sb = [sb.tile([P, nkp], BF16, tag="kt", bufs=D, name=f"kt_sb{d}")
                 for d in range(D)]
        for d in range(D):
            nc.sync.dma_start(out=kt_sb[d], in_=kt[d * P:(d + 1) * P, :])
        # V resident (natural layout): v_sb[j] is [128(key), DV] bf16
        v_sb = [sb.tile([P, DV_], BF16, tag="v", bufs=JS, name=f"v_sb{j}")
                for j in range(JS)]
        for j in range(JS):
            nc.sync.dma_start(out=v_sb[j], in_=v[j * P:(j + 1) * P, :])

        # ---------------- per query block ----------------
        for it in range(NI):
            # q'^T projection for this query block: qt_sb[d] = [128(feat), IW]
            qin = []
            for c in range(C):
                t = sb.tile([P, IW], BF16, tag="xin", bufs=16, name=f"qin{it}_{c}")
                nc.sync.dma_start(out=t, in_=qt[c * P:(c + 1) * P, it * IW:(it + 1) * IW])
                qin.append(t)
            qt_sb = []
            for d in range(D):
                pp = psp.tile([P, IW], F32, tag="ps", bufs=PSB, name=f"ppq{it}_{d}")
                for c in range(C):
                    nc.tensor.matmul(pp, wqk_sb[:, c, d * P:(d + 1) * P], qin[c],
                                     start=(c == 0), stop=(c == C - 1))
                qtile = sb.tile([P, IW], BF16, tag="qt", bufs=D, name=f"qt{it}_{d}")
                nc.scalar.activation(out=qtile, in_=pp, func=AF.Copy)
                qt_sb.append(qtile)

            # scores^T + exp (bias & scale fused): et[j] = [128(key), IW] bf16
            et = []
            for j in range(JS):
                pp = psp.tile([P, IW], F32, tag="ps", bufs=PSB, name=f"pps{it}_{j}")
                for d in range(D):
                    nc.tensor.matmul(pp, kt_sb[d][:, j * P:(j + 1) * P], qt_sb[d],
                                     start=(d == 0), stop=(d == D - 1))
                e_t = sb.tile([P, IW], BF16, tag="et", bufs=JS, name=f"et{it}_{j}")
                nc.scalar.activation(out=e_t, in_=pp, func=AF.Exp, scale=scale,
                                     bias=mb_sb[:, j:j + 1])
                et.append(e_t)

            # softmax denominator, partition-replicated: den[p, i] = sum_j E[i, j]
            ppd = psp.tile([P, IW], F32, tag="ps", bufs=PSB, name=f"ppd{it}")
            for j in range(JS):
                nc.tensor.matmul(ppd, ones, et[j], start=(j == 0), stop=(j == JS - 1))
            recip = sb.tile([P, IW], F32, tag="recip", bufs=2, name=f"recip{it}")
            nc.vector.reciprocal(recip, ppd)

            # attention output (transposed, normalized): ot[d] = [128(feat), IW] bf16
            ot = []
            for d in range(D):
                pp = psp.tile([P, IW], F32, tag="ps", bufs=PSB, name=f"ppo{it}_{d}")
                for j in range(JS):
                    nc.tensor.matmul(pp, v_sb[j][:, d * P:(d + 1) * P], et[j],
                                     start=(j == 0), stop=(j == JS - 1))
                o_t = sb.tile([P, IW], BF16, tag="ot", bufs=D, name=f"ot{it}_{d}")
                nc.vector.tensor_mul(o_t, pp, recip)
                ot.append(o_t)

            # output projection + bias + layernorm, one 128-row slab at a time
            for s in range(IS):
                ysb = sb.tile([P, DV_], F32, tag="y", bufs=4, name=f"y{it}_{s}")
                pps = [psp.tile([P, EW], F32, tag="ps", bufs=PSB,
                                name=f"ppy{it}_{s}_{e}") for e in range(NE)]
                for d in range(D):
                    for e in range(NE):
                        nc.tensor.matmul(pps[e], ot[d][:, s * P:(s + 1) * P],
                                         wvo_sb[:, d, e * EW:(e + 1) * EW],
                                         start=(d == 0), stop=(d == D - 1))
                for e in range(NE):
                    nc.scalar.activation(out=ysb[:, e * EW:(e + 1) * EW],
                                         in_=pps[e], func=AF.Copy)
                nc.vector.tensor_add(ysb, ysb, boe_b)

                stats = sb.tile([P, NB, 6], F32, tag="st", bufs=4, name=f"st{it}_{s}")
                for e in range(NB):
                    nc.vector.bn_stats(out=stats[:, e, :], in_=ysb[:, e * BW:(e + 1) * BW])
                mv = sb.tile([P, 2], F32, tag="mv", bufs=4, name=f"mv{it}_{s}")
                nc.vector.bn_aggr(out=mv, in_=stats)
                std = sb.tile([P, 1], F32, tag="std", bufs=4, name=f"std{it}_{s}")
                nc.scalar.activation(out=std, in_=mv[:, 1:2], func=AF.Sqrt,
                                     bias=eps_sb)
                rstd = sb.tile([P, 1], F32, tag="rstd", bufs=4, name=f"rstd{it}_{s}")
                nc.vector.reciprocal(rstd, std)
                nmr = sb.tile([P, 1], F32, tag="nmr", bufs=4, name=f"nmr{it}_{s}")
                nc.vector.tensor_mul(nmr, mv[:, 0:1], rstd)
                nc.vector.tensor_scalar_mul(nmr, nmr, -1.0)
                nc.scalar.activation(out=ysb, in_=ysb, func=AF.Identity, scale=rstd,
                                     bias=nmr)
                nc.vector.tensor_mul(ysb, ysb, g0_b)
                nc.vector.tensor_add(ysb, ysb, b0_b)
                r0 = it * IW + s * P
                nc.gpsimd.dma_start(out=out[r0:r0 + P, :], in_=ysb)


def build_nc(nq=NQ, nk=1152, dq=DQ, dv=DV, repeat=1, blk=512, hw_loop=0):
    nc = bacc.Bacc("TRN2", target_bir_lowering=False, debug=False)
    ins = {
        "qt": nc.dram_tensor("qt", [dq, nq], BF16, kind="ExternalInput").ap(),
        "kt": nc.dram_tensor("kt", [dq, nk], BF16, kind="ExternalInput").ap(),
        "v": nc.dram_tensor("v", [nk, dv], BF16, kind="ExternalInput").ap(),
        "mb": nc.dram_tensor("mb", [nk], F32, kind="ExternalInput").ap(),
        "wqk": nc.dram_tensor("wqk", [dq, dv], BF16, kind="ExternalInput").ap(),
        "wvo": nc.dram_tensor("wvo", [dv, dv], BF16, kind="ExternalInput").ap(),
        "boe": nc.dram_tensor("boe", [dv], F32, kind="ExternalInput").ap(),
        "g0": nc.dram_tensor("g0", [dv], F32, kind="ExternalInput").ap(),
        "b0": nc.dram_tensor("b0", [dv], F32, kind="ExternalInput").ap(),
    }
    outs = {"out": nc.dram_tensor("out", [nq, dv], F32, kind="ExternalOutput").ap()}
    with tile.TileContext(nc) as tc:
        if hw_loop:
            with tc.For_i(0, hw_loop, 1):
                attention_body(tc, outs, ins, nk, blk=blk)
        else:
            for _ in range(repeat):
                attention_body(tc, outs, ins, nk, blk=blk)
    nc.compile()
    return nc


_NC_CACHE = {}


def make_in_maps(Q, K, V, pad_mask, Wq, bq, Wk, bk, Wv, bv, Wo, bo, g0, beta0):
    """Host-side prep: param-only weight folds + active-key gather.

    Returns (in_maps, nkp) where nkp is the shared padded active-key count
    (multiple of 128) the kernel must be built for.
    """
    bf16 = ml_dtypes.bfloat16
    f32 = np.float32
    Q, K, V = np.asarray(Q, f32), np.asarray(K, f32), np.asarray(V, f32)
    pad_mask = np.asarray(pad_mask)
    Wq, Wk, Wv, Wo = (np.asarray(w, f32) for w in (Wq, Wk, Wv, Wo))
    bq, bv, bo = np.asarray(bq, f32), np.asarray(bv, f32), np.asarray(bo, f32)
    g0, beta0 = np.asarray(g0, f32), np.asarray(beta0, f32)

    scale = f32(1.0 / np.sqrt(DV))
    shared = {
        "wqk": (Wq @ Wk.T).astype(bf16),
        "wvo": (Wv @ Wo).astype(bf16),
        "boe": (bv @ Wo + bo).astype(f32),
        "g0": g0, "b0": beta0,
    }
    wkbq = Wk @ bq  # per-key score bias direction (zero when bq == 0)

    act = pad_mask[:, 0, :] != 0
    n_act = act.sum(axis=1)
    nkp = max(P, int(-(-int(n_act.max()) // P) * P))

    in_maps = []
    for b in range(Q.shape[0]):
        idx = np.nonzero(act[b])[0]
        na = idx.size
        Kb, Vb = K[b][idx], V[b][idx]
        ktp = np.zeros((DQ, nkp), bf16)
        ktp[:, :na] = Kb.T.astype(bf16)
        vp = np.zeros((nkp, DV), bf16)
        vp[:na] = Vb.astype(bf16)
        mb = np.full((nkp,), -1e5, f32)
        mb[:na] = scale * (Kb @ wkbq)
        m = dict(shared)
        m["qt"] = Q[b].T.astype(bf16)
        m["kt"] = ktp
        m["v"] = vp
        m["mb"] = mb
        in_maps.append(m)
    return in_maps, nkp


def kernel(Q, K, V, pad_mask, Wq, bq, Wk, bk, Wv, bv, Wo, bo, g0, beta0):
    in_maps, nkp = make_in_maps(Q, K, V, pad_mask, Wq, bq, Wk, bk, Wv, bv,
                                Wo, bo, g0, beta0)
    if nkp not in _NC_CACHE:
        _NC_CACHE[nkp] = build_nc(nk=nkp)
    nc = _NC_CACHE[nkp]
    res = run_bass_kernel_spmd(nc, in_maps, core_ids=list(range(N_CORES)))
    return np.stack([res.results[c]["out"] for c in range(N_CORES)], axis=0)


# revision 9
# speedup vs baseline: 1.7621x; 1.7621x over previous
"""Trainium2 Bass kernel for batched single-head attention + output projection + layernorm.

Reference computation (per batch element b):
    q = Q@Wq + bq ; k = K@Wk + bk ; v = V@Wv + bv
    S = q k^T / sqrt(DV) ; S[pad_mask==0] = -1e9 ; P = softmax(S)
    out = LN(P v @ Wo + bo; g0, beta0)

Sharding: data-parallel over batch B=8 across the 8 NeuronCores (one batch
element per core, no collectives).

Math folds (exact):
  - q/k only feed the scores: S = (QWq+bq)(KWk+bk)^T. The bk terms are
    constant per query row -> softmax invariant -> drop. Fold
    Wqk = Wq @ Wk^T (host, param-only) so S = Q Wqk K^T + (K @ (Wk bq))^T.
    The bq term is a per-key bias folded into the exp bias below. This
    removes the K projection matmul entirely.
  - softmax rows sum to 1, so out = P (V Wv + bv) Wo + bo
    = P V (Wv Wo) + (bv Wo + bo). Fold Wvo = Wv @ Wo and
    boe = bv @ Wo + bo (host, param-only). Removes the V projection.
  - pad_mask zeroes keys *exactly*: exp(-1e5 + s) underflows to 0 in f32,
    matching the reference's softmax(where(mask==0, -1e9)) which also
    yields exactly-0 weights. So masked keys are gathered away on the
    host: K/V are repacked to only the active keys (padded with zero
    rows + -1e5 bias to a multiple of 128 shared by all cores). With a
    ~50% mask this halves the two NK-sized matmuls.
  - softmax normalization is deferred: O_unnorm = E@V with E = exp(S'),
    normalized by the row-sum computed with a ones-matmul (partition-
    replicated), applied when copying O out of PSUM.
  - scale 1/sqrt(DV) and the per-key bias fuse into the Exp activation:
    E^T = exp(S^T * scale + mbias[j]).

Layout strategy: host passes Q^T/K^T (bf16, key-gathered) and V natural
(key-gathered) so every matmul has its contraction dim on SBUF
partitions; attention runs in transposed score layout (S^T[j,i])
end-to-end, which makes the per-key bias a per-partition bias and
produces the final output in natural [token, feature] layout with zero
on-device transposes.

NOTE: if some batch row has *zero* active keys the reference degenerates
to a uniform softmax over all 2048 keys; this kernel does not reproduce
that (probability 2^-2048 under the spec's random mask).
"""

import numpy as np
import ml_dtypes

import concourse.bass as bass
import concourse.bacc as bacc
import concourse.tile as tile
from concourse import mybir
from concourse.bass_utils import run_bass_kernel_spmd

BF16 = mybir.dt.bfloat16
F32 = mybir.dt.float32
AF = mybir.ActivationFunctionType
P = 128
N_CORES = 8
EPS = 1e-5

# Full-problem shapes (hardcoded; the grading harness runs kernel() standalone).
B, NQ, NK, DQ, DV = 8, 2048, 2048, 1024, 1024


def attention_body(tc, outs, ins, nkp, blk=512, skip_out=False,
                   fake_residents=False, fake_qin=False):
    # skip_out/fake_residents/fake_qin are DIAGNOSTIC-ONLY ablations (timing
    # experiments); the graded kernel path never sets them.
    nc = tc.nc
    qt, kt, v = ins["qt"], ins["kt"], ins["v"]
    mb = ins["mb"]
    wqk, wvo = ins["wqk"], ins["wvo"]
    boe, g0, b0 = ins["boe"], ins["g0"], ins["b0"]
    out = outs["out"]

    DQ_, NQ_ = qt.shape
    DV_ = wvo.shape[1]
    C = DQ_ // P          # input-feature 128-chunks (contraction of q proj)
    D = DV_ // P          # projected-feature 128-chunks
    JS = nkp // P         # active-key 128-chunks
    IW = min(blk, NQ_)    # query block width (psum free dim)
    EW = min(blk, DV_)    # feature block width
    BW = min(512, DV_)    # bn_stats chunk width
    NB = DV_ // BW        # bn_stats chunks
    PSB = 8               # psum slots (8 banks total)
    NI = NQ_ // IW        # query blocks
    NE = DV_ // EW        # feature blocks
    IS = IW // P          # query 128-chunks per query block
    scale = float(DV_) ** -0.5

    with tc.tile_pool(name="sb", bufs=1) as sb, \
         tc.tile_pool(name="psp", bufs=1, space="PSUM") as psp:

        # ---------------- constants + resident tensors ----------------
        ones = sb.tile([P, P], BF16, tag="ones", bufs=1, name="ones")
        nc.vector.memset(ones, 1.0)
        eps_sb = sb.tile([P, 1], F32, tag="eps", bufs=1, name="eps_sb")
        nc.vector.memset(eps_sb, EPS)
        mb_sb = sb.tile([P, JS], F32, tag="mb", bufs=1, name="mb_sb")
        if fake_residents:
            nc.vector.memset(mb_sb, 0.0)
        else:
            nc.gpsimd.dma_start(out=mb_sb, in_=mb.rearrange("(j p) -> p j", p=P))

        def bcast(ap, nm):
            t = sb.tile([P, DV_], F32, tag=nm, bufs=1, name=nm)
            if fake_residents:
                nc.vector.memset(t, 0.0)
                return t
            nc.gpsimd.dma_start(
                out=t,
                in_=bass.AP(tensor=ap.tensor, offset=ap.offset,
                            ap=[[0, P]] + [list(a) for a in ap.ap]),
            )
            return t

        boe_b = bcast(boe, "boe_b")
        g0_b = bcast(g0, "g0_b")
        b0_b = bcast(b0, "b0_b")

        wqk_sb = sb.tile([P, C, DV_], BF16, tag="wqk", bufs=1, name="wqk_sb")
        wvo_sb = sb.tile([P, D, DV_], BF16, tag="wvo", bufs=1, name="wvo_sb")
        # K^T resident: kt_sb[d] is [128(feat), nkp] bf16 (raw, no projection)
        kt_sb = [sb.tile([P, nkp], BF16, tag="kt", bufs=D, name=f"kt_sb{d}")
                 for d in range(D)]
        # V resident (natural layout): v_sb[j] is [128(key), DV] bf16
        v_sb = [sb.tile([P, DV_], BF16, tag="v", bufs=JS, name=f"v_sb{j}")
                for j in range(JS)]
        if fake_residents:
            nc.vector.memset(wqk_sb, 0.0)
            nc.vector.memset(wvo_sb, 0.0)
            for d in range(D):
                nc.vector.memset(kt_sb[d], 0.0)
            for j in range(JS):
                nc.vector.memset(v_sb[j], 0.0)
        else:
            for c in range(C):
                nc.sync.dma_start(out=wqk_sb[:, c, :], in_=wqk[c * P:(c + 1) * P, :])
            for d in range(D):
                nc.sync.dma_start(out=wvo_sb[:, d, :], in_=wvo[d * P:(d + 1) * P, :])
            for d in range(D):
                nc.sync.dma_start(out=kt_sb[d], in_=kt[d * P:(d + 1) * P, :])
            for j in range(JS):
                nc.sync.dma_start(out=v_sb[j], in_=v[j * P:(j + 1) * P, :])

        # ---------------- per query block ----------------
        for it in range(NI):
            # q'^T projection for this query block: qt_sb[d] = [128(feat), IW]
            qin = []
            for c in range(C):
                t = sb.tile([P, IW], BF16, tag="xin", bufs=16, name=f"qin{it}_{c}")
                if fake_qin:
                    nc.vector.memset(t, 0.25)
                else:
                    nc.sync.dma_start(out=t, in_=qt[c * P:(c + 1) * P, it * IW:(it + 1) * IW])
                qin.append(t)
            qt_sb = []
            for d in range(D):
                pp = psp.tile([P, IW], F32, tag="ps", bufs=PSB, name=f"ppq{it}_{d}")
                for c in range(C):
                    nc.tensor.matmul(pp, wqk_sb[:, c, d * P:(d + 1) * P], qin[c],
                                     start=(c == 0), stop=(c == C - 1))
                qtile = sb.tile([P, IW], BF16, tag="qt", bufs=D, name=f"qt{it}_{d}")
                nc.scalar.activation(out=qtile, in_=pp, func=AF.Copy)
                qt_sb.append(qtile)

            # scores^T + exp (bias & scale fused): et[j] = [128(key), IW] bf16
            et = []
            for j in range(JS):
                pp = psp.tile([P, IW], F32, tag="ps", bufs=PSB, name=f"pps{it}_{j}")
                for d in range(D):
                    nc.tensor.matmul(pp, kt_sb[d][:, j * P:(j + 1) * P], qt_sb[d],
                                     start=(d == 0), stop=(d == D - 1))
                e_t = sb.tile([P, IW], BF16, tag="et", bufs=JS, name=f"et{it}_{j}")
                nc.scalar.activation(out=e_t, in_=pp, func=AF.Exp, scale=scale,
                                     bias=mb_sb[:, j:j + 1])
                et.append(e_t)

            # softmax denominator, partition-replicated: den[p, i] = sum_j E[i, j]
            ppd = psp.tile([P, IW], F32, tag="ps", bufs=PSB, name=f"ppd{it}")
            for j in range(JS):
                nc.tensor.matmul(ppd, ones, et[j], start=(j == 0), stop=(j == JS - 1))
            recip = sb.tile([P, IW], F32, tag="recip", bufs=2, name=f"recip{it}")
            nc.vector.reciprocal(recip, ppd)

            # attention output (transposed, normalized): ot[d] = [128(feat), IW] bf16
            ot = []
            for d in range(D):
                pp = psp.tile([P, IW], F32, tag="ps", bufs=PSB, name=f"ppo{it}_{d}")
                for j in range(JS):
                    nc.tensor.matmul(pp, v_sb[j][:, d * P:(d + 1) * P], et[j],
                                     start=(j == 0), stop=(j == JS - 1))
                o_t = sb.tile([P, IW], BF16, tag="ot", bufs=D, name=f"ot{it}_{d}")
                nc.vector.tensor_mul(o_t, pp, recip)
                ot.append(o_t)

            # output projection + bias + layernorm, one 128-row slab at a time
            for s in range(IS):
                ysb = sb.tile([P, DV_], F32, tag="y", bufs=4, name=f"y{it}_{s}")
                pps = [psp.tile([P, EW], F32, tag="ps", bufs=PSB,
                                name=f"ppy{it}_{s}_{e}") for e in range(NE)]
                for d in range(D):
                    for e in range(NE):
                        nc.tensor.matmul(pps[e], ot[d][:, s * P:(s + 1) * P],
                                         wvo_sb[:, d, e * EW:(e + 1) * EW],
                                         start=(d == 0), stop=(d == D - 1))
                for e in range(NE):
                    nc.scalar.activation(out=ysb[:, e * EW:(e + 1) * EW],
                                         in_=pps[e], func=AF.Copy)
                nc.vector.tensor_add(ysb, ysb, boe_b)

                stats = sb.tile([P, NB, 6], F32, tag="st", bufs=4, name=f"st{it}_{s}")
                for e in range(NB):
                    nc.vector.bn_stats(out=stats[:, e, :], in_=ysb[:, e * BW:(e + 1) * BW])
                mv = sb.tile([P, 2], F32, tag="mv", bufs=4, name=f"mv{it}_{s}")
                nc.vector.bn_aggr(out=mv, in_=stats)
                std = sb.tile([P, 1], F32, tag="std", bufs=4, name=f"std{it}_{s}")
                nc.scalar.activation(out=std, in_=mv[:, 1:2], func=AF.Sqrt,
                                     bias=eps_sb)
                rstd = sb.tile([P, 1], F32, tag="rstd", bufs=4, name=f"rstd{it}_{s}")
                nc.vector.reciprocal(rstd, std)
                nmr = sb.tile([P, 1], F32, tag="nmr", bufs=4, name=f"nmr{it}_{s}")
                nc.vector.tensor_mul(nmr, mv[:, 0:1], rstd)
                nc.vector.tensor_scalar_mul(nmr, nmr, -1.0)
                nc.scalar.activation(out=ysb, in_=ysb, func=AF.Identity, scale=rstd,
                                     bias=nmr)
                nc.vector.tensor_mul(ysb, ysb, g0_b)
                nc.vector.tensor_add(ysb, ysb, b0_b)
                r0 = it * IW + s * P
                if skip_out:
                    nc.gpsimd.dma_start(out=out[r0:r0 + P, 0:8], in_=ysb[:, 0:8])
                else:
                    nc.gpsimd.dma_start(out=out[r0:r0 + P, :], in_=ysb)


def build_nc(nq=NQ, nk=1152, dq=DQ, dv=DV, repeat=1, blk=512, hw_loop=0,
             **body_kwargs):
    nc = bacc.Bacc("TRN2", target_bir_lowering=False, debug=False)
    ins = {
        "qt": nc.dram_tensor("qt", [dq, nq], BF16, kind="ExternalInput").ap(),
        "kt": nc.dram_tensor("kt", [dq, nk], BF16, kind="ExternalInput").ap(),
        "v": nc.dram_tensor("v", [nk, dv], BF16, kind="ExternalInput").ap(),
        "mb": nc.dram_tensor("mb", [nk], F32, kind="ExternalInput").ap(),
        "wqk": nc.dram_tensor("wqk", [dq, dv], BF16, kind="ExternalInput").ap(),
        "wvo": nc.dram_tensor("wvo", [dv, dv], BF16, kind="ExternalInput").ap(),
        "boe": nc.dram_tensor("boe", [dv], F32, kind="ExternalInput").ap(),
        "g0": nc.dram_tensor("g0", [dv], F32, kind="ExternalInput").ap(),
        "b0": nc.dram_tensor("b0", [dv], F32, kind="ExternalInput").ap(),
    }
    outs = {"out": nc.dram_tensor("out", [nq, dv], F32, kind="ExternalOutput").ap()}
    with tile.TileContext(nc) as tc:
        if hw_loop:
            with tc.For_i(0, hw_loop, 1):
                attention_body(tc, outs, ins, nk, blk=blk, **body_kwargs)
        else:
            for _ in range(repeat):
                attention_body(tc, outs, ins, nk, blk=blk, **body_kwargs)
    nc.compile()
    return nc


_NC_CACHE = {}


def make_in_maps(Q, K, V, pad_mask, Wq, bq, Wk, bk, Wv, bv, Wo, bo, g0, beta0):
    """Host-side prep: param-only weight folds + active-key gather.

    Returns (in_maps, nkp) where nkp is the shared padded active-key count
    (multiple of 128) the kernel must be built for.
    """
    bf16 = ml_dtypes.bfloat16
    f32 = np.float32
    Q, K, V = np.asarray(Q, f32), np.asarray(K, f32), np.asarray(V, f32)
    pad_mask = np.asarray(pad_mask)
    Wq, Wk, Wv, Wo = (np.asarray(w, f32) for w in (Wq, Wk, Wv, Wo))
    bq, bv, bo = np.asarray(bq, f32), np.asarray(bv, f32), np.asarray(bo, f32)
    g0, beta0 = np.asarray(g0, f32), np.asarray(beta0, f32)

    scale = f32(1.0 / np.sqrt(DV))
    shared = {
        "wqk": (Wq @ Wk.T).astype(bf16),
        "wvo": (Wv @ Wo).astype(bf16),
        "boe": (bv @ Wo + bo).astype(f32),
        "g0": g0, "b0": beta0,
    }
    wkbq = Wk @ bq  # per-key score bias direction (zero when bq == 0)

    act = pad_mask[:, 0, :] != 0
    n_act = act.sum(axis=1)
    nkp = max(P, int(-(-int(n_act.max()) // P) * P))

    in_maps = []
    for b in range(Q.shape[0]):
        idx = np.nonzero(act[b])[0]
        na = idx.size
        Kb, Vb = K[b][idx], V[b][idx]
        ktp = np.zeros((DQ, nkp), bf16)
        ktp[:, :na] = Kb.T.astype(bf16)
        vp = np.zeros((nkp, DV), bf16)
        vp[:na] = Vb.astype(bf16)
        mb = np.full((nkp,), -1e5, f32)
        mb[:na] = scale * (Kb @ wkbq)
        m = dict(shared)
        m["qt"] = Q[b].T.astype(bf16)
        m["kt"] = ktp
        m["v"] = vp
        m["mb"] = mb
        in_maps.append(m)
    return in_maps, nkp


def kernel(Q, K, V, pad_mask, Wq, bq, Wk, bk, Wv, bv, Wo, bo, g0, beta0):
    in_maps, nkp = make_in_maps(Q, K, V, pad_mask, Wq, bq, Wk, bk, Wv, bv,
                                Wo, bo, g0, beta0)
    if nkp not in _NC_CACHE:
        _NC_CACHE[nkp] = build_nc(nk=nkp)
    nc = _NC_CACHE[nkp]
    res = run_bass_kernel_spmd(nc, in_maps, core_ids=list(range(N_CORES)))
    return np.stack([res.results[c]["out"] for c in range(N_CORES)], axis=0)


# revision 11
# speedup vs baseline: 20.5475x; 11.6605x over previous
"""Trainium2 Bass kernel for batched single-head attention + output projection + layernorm.

Reference computation (per batch element b):
    q = Q@Wq + bq ; k = K@Wk + bk ; v = V@Wv + bv
    S = q k^T / sqrt(DV) ; S[pad_mask==0] = -1e9 ; P = softmax(S)
    out = LN(P v @ Wo + bo; g0, beta0)

Sharding: data-parallel over batch B=8 across the 8 NeuronCores (one batch
element per core, no collectives).

Math folds (exact):
  - q/k only feed the scores: S = (QWq+bq)(KWk+bk)^T. The bk terms are
    constant per query row -> softmax invariant -> drop. Fold
    Wqk = Wq @ Wk^T (host, param-only) so S = Q Wqk K^T + (K @ (Wk bq))^T.
    The bq term is a per-key bias folded into the exp bias below. This
    removes the K projection matmul entirely.
  - softmax rows sum to 1, so out = P (V Wv + bv) Wo + bo
    = P V (Wv Wo) + (bv Wo + bo). Fold Wvo = Wv @ Wo and
    boe = bv @ Wo + bo (host, param-only). Removes the V projection.
  - pad_mask zeroes keys *exactly*: exp(-1e5 + s) underflows to 0 in f32,
    matching the reference's softmax(where(mask==0, -1e9)) which also
    yields exactly-0 weights. So masked keys are gathered away on the
    host: K/V are repacked to only the active keys (padded with zero
    rows + -1e5 bias to a multiple of 128 shared by all cores). With a
    ~50% mask this halves the two NK-sized matmuls.
  - softmax normalization is deferred: O_unnorm = E@V with E = exp(S'),
    normalized by the row-sum computed with a ones-matmul (partition-
    replicated), applied when copying O out of PSUM.
  - scale 1/sqrt(DV) and the per-key bias fuse into the Exp activation:
    E^T = exp(S^T * scale + mbias[j]).

Layout strategy: host passes Q^T/K^T (bf16, key-gathered) and V natural
(key-gathered) so every matmul has its contraction dim on SBUF
partitions; attention runs in transposed score layout (S^T[j,i])
end-to-end, which makes the per-key bias a per-partition bias and
produces the final output in natural [token, feature] layout with zero
on-device transposes.

NOTE: if some batch row has *zero* active keys the reference degenerates
to a uniform softmax over all 2048 keys; this kernel does not reproduce
that (probability 2^-2048 under the spec's random mask).
"""

import numpy as np
import ml_dtypes

import concourse.bass as bass
import concourse.bacc as bacc
import concourse.tile as tile
from concourse import mybir
from concourse.bass_utils import run_bass_kernel_spmd

BF16 = mybir.dt.bfloat16
F32 = mybir.dt.float32
AF = mybir.ActivationFunctionType
P = 128
N_CORES = 8
EPS = 1e-5

# Full-problem shapes (hardcoded; the grading harness runs kernel() standalone).
B, NQ, NK, DQ, DV = 8, 2048, 2048, 1024, 1024


def attention_body(tc, outs, ins, nkp, blk=512, skip_out=False,
                   fake_residents=False, fake_qin=False):
    # skip_out/fake_residents/fake_qin are DIAGNOSTIC-ONLY ablations (timing
    # experiments); the graded kernel path never sets them.
    nc = tc.nc
    qt, kt, v = ins["qt"], ins["kt"], ins["v"]
    mb = ins["mb"]
    wqk, wvo = ins["wqk"], ins["wvo"]
    boe, g0, b0 = ins["boe"], ins["g0"], ins["b0"]
    out = outs["out"]

    DQ_, NQ_ = qt.shape
    DV_ = wvo.shape[1]
    C = DQ_ // P          # input-feature 128-chunks (contraction of q proj)
    D = DV_ // P          # projected-feature 128-chunks
    JS = nkp // P         # active-key 128-chunks
    IW = min(blk, NQ_)    # query block width (psum free dim)
    EW = min(blk, DV_)    # feature block width
    BW = min(512, DV_)    # bn_stats chunk width
    NB = DV_ // BW        # bn_stats chunks
    PSB = 8               # psum slots (8 banks total)
    NI = NQ_ // IW        # query blocks
    NE = DV_ // EW        # feature blocks
    IS = IW // P          # query 128-chunks per query block
    scale = float(DV_) ** -0.5

    with tc.tile_pool(name="sb", bufs=1) as sb, \
         tc.tile_pool(name="psp", bufs=1, space="PSUM") as psp:

        # ---------------- constants + resident tensors ----------------
        ones = sb.tile([P, P], BF16, tag="ones", bufs=1, name="ones")
        nc.vector.memset(ones, 1.0)
        eps_sb = sb.tile([P, 1], F32, tag="eps", bufs=1, name="eps_sb")
        nc.vector.memset(eps_sb, EPS)
        mb_sb = sb.tile([P, JS], F32, tag="mb", bufs=1, name="mb_sb")
        if fake_residents:
            nc.vector.memset(mb_sb, 0.0)
        else:
            nc.gpsimd.dma_start(out=mb_sb, in_=mb.rearrange("(j p) -> p j", p=P))

        def bcast(ap, nm):
            t = sb.tile([P, DV_], F32, tag=nm, bufs=1, name=nm)
            if fake_residents:
                nc.vector.memset(t, 0.0)
                return t
            nc.gpsimd.dma_start(
                out=t,
                in_=bass.AP(tensor=ap.tensor, offset=ap.offset,
                            ap=[[0, P]] + [list(a) for a in ap.ap]),
            )
            return t

        boe_b = bcast(boe, "boe_b")
        g0_b = bcast(g0, "g0_b")
        b0_b = bcast(b0, "b0_b")

        wqk_sb = sb.tile([P, C, DV_], BF16, tag="wqk", bufs=1, name="wqk_sb")
        wvo_sb = sb.tile([P, D, DV_], BF16, tag="wvo", bufs=1, name="wvo_sb")
        # K^T resident: kt_sb[d] is [128(feat), nkp] bf16 (raw, no projection)
        kt_sb = [sb.tile([P, nkp], BF16, tag="kt", bufs=D, name=f"kt_sb{d}")
                 for d in range(D)]
        # V resident (natural layout): v_sb[j] is [128(key), DV] bf16
        v_sb = [sb.tile([P, DV_], BF16, tag="v", bufs=JS, name=f"v_sb{j}")
                for j in range(JS)]
        # DMA issue order is tuned for the critical path: wqk (first consumer)
        # -> all q inputs -> kt -> v -> wvo (latest consumer). All on the sync
        # HWDGE ring, which drains in FIFO order.
        qin_all = [[sb.tile([P, IW], BF16, tag="xin", bufs=4 * C,
                            name=f"qin{it}_{c}") for c in range(C)]
                   for it in range(NI)]
        if fake_residents:
            nc.vector.memset(wqk_sb, 0.0)
            nc.vector.memset(wvo_sb, 0.0)
            for d in range(D):
                nc.vector.memset(kt_sb[d], 0.0)
            for j in range(JS):
                nc.vector.memset(v_sb[j], 0.0)
        else:
            for c in range(C):
                nc.sync.dma_start(out=wqk_sb[:, c, :], in_=wqk[c * P:(c + 1) * P, :])
        if fake_qin:
            for it in range(NI):
                for c in range(C):
                    nc.vector.memset(qin_all[it][c], 0.25)
        else:
            for it in range(NI):
                for c in range(C):
                    nc.sync.dma_start(
                        out=qin_all[it][c],
                        in_=qt[c * P:(c + 1) * P, it * IW:(it + 1) * IW])
        if not fake_residents:
            for d in range(D):
                nc.sync.dma_start(out=kt_sb[d], in_=kt[d * P:(d + 1) * P, :])
            for j in range(JS):
                nc.sync.dma_start(out=v_sb[j], in_=v[j * P:(j + 1) * P, :])
            for d in range(D):
                nc.sync.dma_start(out=wvo_sb[:, d, :], in_=wvo[d * P:(d + 1) * P, :])

        # ---------------- per query block ----------------
        for it in range(NI):
            # q'^T projection for this query block: qt_sb[d] = [128(feat), IW]
            qin = qin_all[it]
            qt_sb = []
            for d in range(D):
                pp = psp.tile([P, IW], F32, tag="ps", bufs=PSB, name=f"ppq{it}_{d}")
                for c in range(C):
                    nc.tensor.matmul(pp, wqk_sb[:, c, d * P:(d + 1) * P], qin[c],
                                     start=(c == 0), stop=(c == C - 1))
                qtile = sb.tile([P, IW], BF16, tag="qt", bufs=D, name=f"qt{it}_{d}")
                nc.scalar.activation(out=qtile, in_=pp, func=AF.Copy)
                qt_sb.append(qtile)

            # scores^T + exp (bias & scale fused): et[j] = [128(key), IW] bf16
            et = []
            for j in range(JS):
                pp = psp.tile([P, IW], F32, tag="ps", bufs=PSB, name=f"pps{it}_{j}")
                for d in range(D):
                    nc.tensor.matmul(pp, kt_sb[d][:, j * P:(j + 1) * P], qt_sb[d],
                                     start=(d == 0), stop=(d == D - 1))
                e_t = sb.tile([P, IW], BF16, tag="et", bufs=JS, name=f"et{it}_{j}")
                nc.scalar.activation(out=e_t, in_=pp, func=AF.Exp, scale=scale,
                                     bias=mb_sb[:, j:j + 1])
                et.append(e_t)

            # softmax denominator, partition-replicated: den[p, i] = sum_j E[i, j]
            ppd = psp.tile([P, IW], F32, tag="ps", bufs=PSB, name=f"ppd{it}")
            for j in range(JS):
                nc.tensor.matmul(ppd, ones, et[j], start=(j == 0), stop=(j == JS - 1))
            recip = sb.tile([P, IW], F32, tag="recip", bufs=2, name=f"recip{it}")
            nc.vector.reciprocal(recip, ppd)

            # attention output (transposed, normalized): ot[d] = [128(feat), IW] bf16
            ot = []
            for d in range(D):
                pp = psp.tile([P, IW], F32, tag="ps", bufs=PSB, name=f"ppo{it}_{d}")
                for j in range(JS):
                    nc.tensor.matmul(pp, v_sb[j][:, d * P:(d + 1) * P], et[j],
                                     start=(j == 0), stop=(j == JS - 1))
                o_t = sb.tile([P, IW], BF16, tag="ot", bufs=D, name=f"ot{it}_{d}")
                nc.vector.tensor_mul(o_t, pp, recip)
                ot.append(o_t)

            # output projection + bias + layernorm, one 128-row slab at a time
            for s in range(IS):
                ysb = sb.tile([P, DV_], F32, tag="y", bufs=4, name=f"y{it}_{s}")
                pps = [psp.tile([P, EW], F32, tag="ps", bufs=PSB,
                                name=f"ppy{it}_{s}_{e}") for e in range(NE)]
                for d in range(D):
                    for e in range(NE):
                        nc.tensor.matmul(pps[e], ot[d][:, s * P:(s + 1) * P],
                                         wvo_sb[:, d, e * EW:(e + 1) * EW],
                                         start=(d == 0), stop=(d == D - 1))
                for e in range(NE):
                    nc.scalar.activation(out=ysb[:, e * EW:(e + 1) * EW],
                                         in_=pps[e], func=AF.Copy)
                nc.vector.tensor_add(ysb, ysb, boe_b)

                stats = sb.tile([P, NB, 6], F32, tag="st", bufs=4, name=f"st{it}_{s}")
                for e in range(NB):
                    nc.vector.bn_stats(out=stats[:, e, :], in_=ysb[:, e * BW:(e + 1) * BW])
                mv = sb.tile([P, 2], F32, tag="mv", bufs=4, name=f"mv{it}_{s}")
                nc.vector.bn_aggr(out=mv, in_=stats)
                std = sb.tile([P, 1], F32, tag="std", bufs=4, name=f"std{it}_{s}")
                nc.scalar.activation(out=std, in_=mv[:, 1:2], func=AF.Sqrt,
                                     bias=eps_sb)
                rstd = sb.tile([P, 1], F32, tag="rstd", bufs=4, name=f"rstd{it}_{s}")
                nc.vector.reciprocal(rstd, std)
                nmr = sb.tile([P, 1], F32, tag="nmr", bufs=4, name=f"nmr{it}_{s}")
                nc.vector.tensor_mul(nmr, mv[:, 0:1], rstd)
                nc.vector.tensor_scalar_mul(nmr, nmr, -1.0)
                nc.scalar.activation(out=ysb, in_=ysb, func=AF.Identity, scale=rstd,
                                     bias=nmr)
                nc.vector.tensor_mul(ysb, ysb, g0_b)
                nc.vector.tensor_add(ysb, ysb, b0_b)
                r0 = it * IW + s * P
                if skip_out:
                    nc.gpsimd.dma_start(out=out[r0:r0 + P, 0:8], in_=ysb[:, 0:8])
                else:
                    nc.gpsimd.dma_start(out=out[r0:r0 + P, :], in_=ysb)


def build_nc(nq=NQ, nk=1152, dq=DQ, dv=DV, repeat=1, blk=512, hw_loop=0,
             **body_kwargs):
    nc = bacc.Bacc("TRN2", target_bir_lowering=False, debug=False)
    ins = {
        "qt": nc.dram_tensor("qt", [dq, nq], BF16, kind="ExternalInput").ap(),
        "kt": nc.dram_tensor("kt", [dq, nk], BF16, kind="ExternalInput").ap(),
        "v": nc.dram_tensor("v", [nk, dv], BF16, kind="ExternalInput").ap(),
        "mb": nc.dram_tensor("mb", [nk], F32, kind="ExternalInput").ap(),
        "wqk": nc.dram_tensor("wqk", [dq, dv], BF16, kind="ExternalInput").ap(),
        "wvo": nc.dram_tensor("wvo", [dv, dv], BF16, kind="ExternalInput").ap(),
        "boe": nc.dram_tensor("boe", [dv], F32, kind="ExternalInput").ap(),
        "g0": nc.dram_tensor("g0", [dv], F32, kind="ExternalInput").ap(),
        "b0": nc.dram_tensor("b0", [dv], F32, kind="ExternalInput").ap(),
    }
    outs = {"out": nc.dram_tensor("out", [nq, dv], F32, kind="ExternalOutput").ap()}
    with tile.TileContext(nc) as tc:
        if hw_loop:
            with tc.For_i(0, hw_loop, 1):
                attention_body(tc, outs, ins, nk, blk=blk, **body_kwargs)
        else:
            for _ in range(repeat):
                attention_body(tc, outs, ins, nk, blk=blk, **body_kwargs)
    nc.compile()
    return nc


_NC_CACHE = {}


def make_in_maps(Q, K, V, pad_mask, Wq, bq, Wk, bk, Wv, bv, Wo, bo, g0, beta0):
    """Host-side prep: param-only weight folds + active-key gather.

    Returns (in_maps, nkp) where nkp is the shared padded active-key count
    (multiple of 128) the kernel must be built for.
    """
    bf16 = ml_dtypes.bfloat16
    f32 = np.float32
    Q, K, V = np.asarray(Q, f32), np.asarray(K, f32), np.asarray(V, f32)
    pad_mask = np.asarray(pad_mask)
    Wq, Wk, Wv, Wo = (np.asarray(w, f32) for w in (Wq, Wk, Wv, Wo))
    bq, bv, bo = np.asarray(bq, f32), np.asarray(bv, f32), np.asarray(bo, f32)
    g0, beta0 = np.asarray(g0, f32), np.asarray(beta0, f32)

    scale = f32(1.0 / np.sqrt(DV))
    shared = {
        "wqk": (Wq @ Wk.T).astype(bf16),
        "wvo": (Wv @ Wo).astype(bf16),
        "boe": (bv @ Wo + bo).astype(f32),
        "g0": g0, "b0": beta0,
    }
    wkbq = Wk @ bq  # per-key score bias direction (zero when bq == 0)

    act = pad_mask[:, 0, :] != 0
    n_act = act.sum(axis=1)
    nkp = max(P, int(-(-int(n_act.max()) // P) * P))

    in_maps = []
    for b in range(Q.shape[0]):
        idx = np.nonzero(act[b])[0]
        na = idx.size
        Kb, Vb = K[b][idx], V[b][idx]
        ktp = np.zeros((DQ, nkp), bf16)
        ktp[:, :na] = Kb.T.astype(bf16)
        vp = np.zeros((nkp, DV), bf16)
        vp[:na] = Vb.astype(bf16)
        mb = np.full((nkp,), -1e5, f32)
        mb[:na] = scale * (Kb @ wkbq)
        m = dict(shared)
        m["qt"] = Q[b].T.astype(bf16)
        m["kt"] = ktp
        m["v"] = vp
        m["mb"] = mb
        in_maps.append(m)
    return in_maps, nkp


def kernel(Q, K, V, pad_mask, Wq, bq, Wk, bk, Wv, bv, Wo, bo, g0, beta0):
    in_maps, nkp = make_in_maps(Q, K, V, pad_mask, Wq, bq, Wk, bk, Wv, bv,
                                Wo, bo, g0, beta0)
    if nkp not in _NC_CACHE:
        _NC_CACHE[nkp] = build_nc(nk=nkp)
    nc = _NC_CACHE[nkp]
    res = run_bass_kernel_spmd(nc, in_maps, core_ids=list(range(N_CORES)))
    return np.stack([res.results[c]["out"] for c in range(N_CORES)], axis=0)
